# revision 1
# baseline (speedup 1.0000x reference)
"""Trainium2 Bass kernel for BasePropagationGraphPositionalEncoding.

Computes, for each batch element b:
    out[b] = (sum_k coefs[k] * gr_kernel[b, k]) @ x[b] / sum_k coefs[k]
with coefs[k] = (1 - EPS)^k, EPS = 0.01, K = 9.

Sharding: batch dim B=8 across the 8 NeuronCores (data parallel, no
cross-core communication). Each core streams its 36 MB of gr_kernel slabs
from HBM (the memory-bound term), does the weighted k-sum on VectorE
(fused multiply-accumulate via scalar_tensor_tensor), transposes the
summed [128,128] tiles on TensorE (fp32 transpose mode), and contracts
against x with PSUM-accumulated matmuls.
"""

import sys

if "/opt/trn_rl_repo" not in sys.path:
    sys.path.insert(0, "/opt/trn_rl_repo")

import numpy as np

import concourse.bass as bass
import concourse.mybir as mybir
from concourse import tile
from concourse.bacc import Bacc
from concourse.masks import make_identity
from concourse.bass_utils import run_bass_kernel_spmd

# Problem shapes (hardcoded per the harness contract).
B, K, N, D = 8, 9, 1024, 64
EPS = 0.01
P = 128          # SBUF partitions
NT = N // P      # 8 row/col tiles of the [N, N] kernel

F32 = mybir.dt.float32


def build_bass() -> bass.Bass:
    # Bacc (not plain Bass): its compile() runs generate_event_semaphores /
    # move_matmul_waits_to_ldweights, splitting multi-semaphore waits that
    # the 64B ISA instructions (single EVENTS slot) cannot carry.
    nc = Bacc()

    x_d = nc.dram_tensor("x_b", (N, D), F32, kind="ExternalInput")
    g_d = nc.dram_tensor("g_b", (K, N, N), F32, kind="ExternalInput")
    o_d = nc.dram_tensor("out_b", (N, D), F32, kind="ExternalOutput")

    coefs = (1.0 - EPS) ** np.arange(K, dtype=np.float64)
    w = coefs / coefs.sum()  # fold the 1/sum normalization into the k-sum

    with tile.TileContext(nc) as tc:
        with (
            tc.tile_pool(name="consts", bufs=1) as consts,
            tc.tile_pool(name="gr", bufs=2) as gr_pool,
            tc.tile_pool(name="wk", bufs=2) as wk_pool,
            tc.tile_pool(name="wkt", bufs=4) as wkt_pool,
            tc.tile_pool(name="outp", bufs=2) as out_pool,
            tc.tile_pool(name="ps_t", bufs=4, space=bass.MemorySpace.PSUM) as ps_t,
            tc.tile_pool(name="ps_e", bufs=2, space=bass.MemorySpace.PSUM) as ps_e,
        ):
            # Per-band, per-slab loads: slab k of band i is its own tile and
            # its own contiguous 512 KB DMA. Separate tiles are essential:
            # slab-DMAs into slices of a shared tile get WAW-serialized by
            # Tile (one DMA in flight -> stream drops from 388 to 333 GB/s),
            # and per-slab deps let the VectorE k-sum start as soon as slab 0
            # lands.
            def load_band(i):
                tiles = []
                for k in range(K):
                    g_k = gr_pool.tile([P, N], F32, tag=f"g{k}")
                    nc.sync.dma_start(g_k[:], g_d[k, i * P : (i + 1) * P, :])
                    tiles.append(g_k)
                return tiles

            # Band 0 has no dependency waits, so issue its loads from three
            # engines in parallel (SP + ACT + GpSimd each issue ~0.6 us per
            # DMA) to fill the SDMA queues ~3x faster during the startup
            # ramp. Steady-state bands stay on SP.
            band_tiles = []
            first_issuers = [nc.sync, nc.scalar, nc.gpsimd]
            for k in range(K):
                g_k = gr_pool.tile([P, N], F32, tag=f"g{k}", name=f"g0_{k}")
                first_issuers[k % 3].dma_start(g_k[:], g_d[k, 0:P, :])
                band_tiles.append(g_k)

            # Identity for TensorE transpose (emitted after the band-0 load
            # issues so GpSimd rings the DMA doorbells first). Built by
            # GPSIMD, then copied through VectorE so the first PE transpose
            # waits on a single semaphore (DVE) — Matmult lowering only
            # supports one sync wait.
            ident_raw = consts.tile([P, P], F32)
            make_identity(nc, ident_raw)
            ident = consts.tile([P, P], F32)
            nc.vector.tensor_copy(ident[:], ident_raw[:])

            # x rearranged to [p, chunk, d] so chunk c is a [128, 64] tile
            # with the contraction index m = c*128 + p on partitions.
            # Loaded after band 0's stream is issued — x isn't needed until
            # the first emb matmul.
            x_sb = consts.tile([P, NT, D], F32)
            nc.gpsimd.dma_start(x_sb[:], x_d.rearrange("(c p) d -> p c d", p=P))

            H = N // 2

            for i in range(NT):
                last = i == NT - 1
                g_ts = band_tiles
                if i + 1 < NT:
                    band_tiles = load_band(i + 1)

                # Weighted k-sum on VectorE: wk = sum_k w[k] * slab_k.
                if not last:
                    wk = wk_pool.tile([P, N], F32)
                    nc.vector.tensor_scalar_mul(wk[:], g_ts[0][:], float(w[0]))
                    for k in range(1, K):
                        nc.vector.scalar_tensor_tensor(
                            wk[:],
                            g_ts[k][:],
                            float(w[k]),
                            wk[:],
                            op0=mybir.AluOpType.mult,
                            op1=mybir.AluOpType.add,
                        )

                    def wk_cols(c):
                        return wk[:, c * P : (c + 1) * P]

                else:
                    # Last band: same full-slab DMAs, but the k-sum runs as
                    # two independent half-width chains in separate tiles.
                    # The final transposes for chunks 0-3 then wait on a
                    # 0.69 us half-op instead of the 1.28 us full op,
                    # starting the tail PE chain earlier.
                    wk_h = []
                    for h in range(2):
                        t = wk_pool.tile(
                            [P, H], F32, tag=f"wkh{h}", name=f"wk_h{h}"
                        )
                        wk_h.append(t)
                    for k in range(K):
                        # k=8 (the very last slab) folds in quarter-width
                        # ops so the first transposes can start ~0.6 us
                        # earlier; earlier k's stay at half width to keep
                        # DVE ops under the 1.35 us slab-arrival cadence.
                        nsplit = 4 if k == K - 1 else 2
                        W = N // nsplit
                        for s in range(nsplit):
                            h = s * W // H
                            off = s * W - h * H
                            src = g_ts[k][:, s * W : (s + 1) * W]
                            dst = wk_h[h][:, off : off + W]
                            if k == 0:
                                nc.vector.tensor_scalar_mul(
                                    dst, src, float(w[0])
                                )
                            else:
                                nc.vector.scalar_tensor_tensor(
                                    dst,
                                    src,
                                    float(w[k]),
                                    dst,
                                    op0=mybir.AluOpType.mult,
                                    op1=mybir.AluOpType.add,
                                )

                    def wk_cols(c):
                        return wk_h[c // 4][:, (c % 4) * P : (c % 4 + 1) * P]

                # Transpose the 8 [128,128] tiles of wk on TensorE; each
                # chunk is staged to SBUF by its own 250 ns ACT copy right
                # after its transpose, so the first emb matmul is gated by
                # PE availability, not by a batched copy.
                wkT_sb = wkt_pool.tile([P, NT, P], F32)
                for c in range(NT):
                    wkT_ps = ps_t.tile([P, P], F32)
                    nc.tensor.transpose(wkT_ps[:], wk_cols(c), ident[:])
                    nc.scalar.copy(wkT_sb[:, c, :], wkT_ps[:])

                # emb[i-band] = sum_c wk_tile(i,c) @ x_chunk(c), accumulated
                # in PSUM over the 8 contraction chunks.
                emb_ps = ps_e.tile([P, D], F32)
                for c in range(NT):
                    nc.tensor.matmul(
                        emb_ps[:],
                        wkT_sb[:, c, :],
                        x_sb[:, c, :],
                        start=(c == 0),
                        stop=(c == NT - 1),
                    )

                o_sb = out_pool.tile([P, D], F32)
                nc.scalar.copy(o_sb[:], emb_ps[:])
                nc.gpsimd.dma_start(o_d[i * P : (i + 1) * P, :], o_sb[:])

    nc.compile()
    return nc


_NC = None


def _get_nc() -> bass.Bass:
    global _NC
    if _NC is None:
        _NC = build_bass()
    return _NC


def run(x: np.ndarray, gr_kernel: np.ndarray, **spmd_kwargs):
    """Run the SPMD kernel on cores 0-7; returns BassKernelResults."""
    nc = _get_nc()
    in_maps = [
        {
            "x_b": np.ascontiguousarray(x[b], dtype=np.float32),
            "g_b": np.ascontiguousarray(gr_kernel[b], dtype=np.float32),
        }
        for b in range(B)
    ]
    return run_bass_kernel_spmd(nc, in_maps, core_ids=list(range(B)), **spmd_kwargs)


def kernel(x: np.ndarray, gr_kernel: np.ndarray) -> np.ndarray:
    res = run(np.asarray(x), np.asarray(gr_kernel))
    out = np.stack([res.results[b]["out_b"] for b in range(B)], axis=0)
    return out.astype(np.float32, copy=False)


if __name__ == "__main__":
    rng = np.random.default_rng(0)
    x = rng.standard_normal((B, N, D), dtype=np.float32)
    g = rng.standard_normal((B, K, N, N), dtype=np.float32)
    out = kernel(x, g)
    coefs = (1.0 - EPS) ** np.arange(K)
    wk = np.einsum("k,bknm->bnm", coefs, g)
    ref = np.matmul(wk, x) / coefs.sum()
    err = np.linalg.norm(out - ref) / np.linalg.norm(ref)
    print("self-check rel err:", err)

